# revision 1
# baseline (speedup 1.0000x reference)
"""BiaffineAttention TRN2 kernel.

Full-input contract: kernel(**inputs) takes the unsharded reference inputs
(hidden_states [16,512,1024] f32 + MLP/bilinear params) and returns the full
arc_scores [16,512,512] f32.

Strategy:
- Data-parallel over batch across 8 NeuronCores (2 batches/core).
- All on-chip compute is feature-major (arc/hidden on partitions), so every
  matmul has its contraction dim on partitions and there are no on-chip
  transposes: the host passes x pre-transposed per core and weights
  pre-transposed + zero-padded (arc 500 -> 512).
- The bilinear weight Wb is fused into the head MLP's second layer on the
  host (Wf = w2h.T @ Wb, bf = b2h @ Wb, in float64), removing a whole
  [500x500] GEMM stage from the device.
- Matmuls run in float16 (same 10-bit mantissa as tf32, ~5e-4 relative
  error, but 2-byte operands at full PE issue rate with pipelined weight
  loads). float32r/bf16 variants remain selectable via _CACHE for testing.
"""

import sys

if "/opt/trn_rl_repo" not in sys.path:
    sys.path.insert(0, "/opt/trn_rl_repo")

import numpy as np

import concourse.bacc as bacc
import concourse.mybir as mybir
import concourse.tile as tile
from concourse.bass_utils import run_bass_kernel_spmd

N_CORES = 8
BATCH = 16
SEQ = 512
HIDDEN = 1024
ARC = 500
ARC_P = 512  # arc padded to a multiple of 128

P = 128
B_PER_CORE = BATCH // N_CORES  # 2
R = B_PER_CORE * SEQ  # 1024 rows per core
HK = HIDDEN // P  # 8 hidden k-tiles
AK = ARC_P // P  # 4 arc tiles
RC = R // SEQ  # 2 row chunks of 512

F32 = mybir.dt.float32
F32R = mybir.dt.float32r
BF16 = mybir.dt.bfloat16
F16 = mybir.dt.float16
AF = mybir.ActivationFunctionType

# matmul operand dtypes: (stationary/weight side, moving/activation side)
_DT_MODES = {
    "f32r": (F32R, F32R),
    "bf16": (BF16, BF16),
    "fp16": (F16, F16),
    "mixed": (BF16, F32R),
}
# max moving-operand width: 512 for 4-byte dtypes, 1024 for 2-byte
_MOV_W = {"f32r": 512, "bf16": 512, "fp16": 512, "mixed": 512}

_CACHE = {}


_DEFAULTS = {"dt_mode": "fp16"}


def _cfg(name, default):
    return _CACHE.get(name, _DEFAULTS.get(name, default))


def _emit(nc, tc, aps, loop_n=0):
    import contextlib

    mode = _cfg("dt_mode", "f32r")
    sd, md = _DT_MODES[mode]
    # scores-phase dtypes (both operands are on-chip activations)
    ssd = md if _cfg("scores_f32r", True) else sd
    smd = md

    ctx = contextlib.ExitStack()
    with ctx:
        cpool = ctx.enter_context(tc.tile_pool(name="const", bufs=1))
        apool = ctx.enter_context(tc.tile_pool(name="acts", bufs=1))
        pspool = ctx.enter_context(
            tc.tile_pool(name="psum", bufs=_cfg("ps_bufs", 8), space="PSUM")
        )
        opool = ctx.enter_context(tc.tile_pool(name="outs", bufs=8))

        # ---- constant loads, split per k-tile and ordered by first use so the
        # first matmul group starts as soon as its own slices land
        xT = cpool.tile([P, HK, R], md, tag="xT")
        w1h = cpool.tile([P, HK, ARC_P], sd, tag="w1h")
        w1d = cpool.tile([P, HK, ARC_P], sd, tag="w1d")
        xT_src = aps["xT"].rearrange("(ko p) r -> p ko r", p=P)
        w1h_src = aps["w1hT"].rearrange("(ko p) a -> p ko a", p=P)
        w1d_src = aps["w1dT"].rearrange("(ko p) a -> p ko a", p=P)
        # w1h delivered progressively: first k-tiles on the HWDGE rings for
        # the fastest start, singles then bulk on SWDGE for the rest
        nc.sync.dma_start(w1h[:, 0], w1h_src[:, 0])
        nc.scalar.dma_start(w1h[:, 1], w1h_src[:, 1])
        nc.gpsimd.dma_start(w1h[:, 2], w1h_src[:, 2])
        nc.gpsimd.dma_start(w1h[:, 3], w1h_src[:, 3])
        nc.gpsimd.dma_start(w1h[:, 4:], w1h_src[:, 4:])
        # xT rc0: per-k tiles alternating both HWDGE rings (fast start);
        # xT rc1: two bulk transfers so delivery is not dispatch-bound
        for k in range(HK):
            eng = nc.scalar if k % 2 == 0 else nc.sync
            eng.dma_start(xT[:, k, 0:SEQ], xT_src[:, k, 0:SEQ])
        half = HK // 2
        nc.scalar.dma_start(xT[:, :half, SEQ:R], xT_src[:, :half, SEQ:R])
        nc.sync.dma_start(xT[:, half:, SEQ:R], xT_src[:, half:, SEQ:R])
        biases = cpool.tile([P, 4 * AK + 1], F32, tag="biases")
        nc.gpsimd.dma_start(biases[:], aps["biasesL"])
        b1h = biases[:, 0 * AK : 1 * AK]
        bfh = biases[:, 1 * AK : 2 * AK]
        b1d = biases[:, 2 * AK : 3 * AK]
        b2d = biases[:, 3 * AK : 4 * AK]
        nc.gpsimd.dma_start(w1d[:, :half], w1d_src[:, :half])
        nc.gpsimd.dma_start(w1d[:, half:], w1d_src[:, half:])
        wf = cpool.tile([P, AK, ARC_P], sd, tag="wf")
        nc.gpsimd.dma_start(wf[:], aps["wfT"].rearrange("(ko p) a -> p ko a", p=P))
        w2d = cpool.tile([P, AK, ARC_P], sd, tag="w2d")
        nc.gpsimd.dma_start(w2d[:], aps["w2dT"].rearrange("(ko p) a -> p ko a", p=P))

        h1h = apool.tile([P, AK, R], md, tag="h1h")
        h1d = apool.tile([P, AK, R], md, tag="h1d")
        headWT = apool.tile([P, AK, R], ssd, tag="headWT")
        depT = apool.tile([P, AK, R], smd, tag="depT")

        def l1_half(w1, b1, h1, rc):
            # h1[:, :, rc-half] = relu(w1.T @ xT + b1); all 4 m-tiles sweep
            # each arriving xT k-tile (852ns of matmul per 728ns of DMA feed,
            # so PE stays ahead of the input stream)
            rs = slice(rc * SEQ, (rc + 1) * SEQ)
            pss = [
                pspool.tile([P, SEQ], F32, tag="ps", name=f"l1_{rc}_{m}")
                for m in range(AK)
            ]
            for k in range(HK):
                for m in range(AK):
                    nc.tensor.matmul(
                        pss[m][:],
                        w1[:, k, m * P : (m + 1) * P],
                        xT[:, k, rs],
                        start=(k == 0),
                        stop=(k == HK - 1),
                    )
            for m in range(AK):
                nc.scalar.activation(
                    h1[:, m, rs], pss[m][:], AF.Relu, bias=b1[:, m : m + 1]
                )

        def l2_group(w2, b2, h1, outbuf, rc, m, epi):
            # outbuf[:, m, rc-half] = w2.T @ h1 + b2 at 512 width
            rs = slice(rc * SEQ, (rc + 1) * SEQ)
            ps = pspool.tile([P, SEQ], F32, tag="ps", name=f"ps2l_{rc}_{m}")
            for k in range(AK):
                nc.tensor.matmul(
                    ps[:],
                    w2[:, k, m * P : (m + 1) * P],
                    h1[:, k, rs],
                    start=(k == 0),
                    stop=(k == AK - 1),
                )
            if epi == "dve":
                nc.vector.tensor_tensor(
                    outbuf[:, m, rs],
                    ps[:],
                    b2[:, m : m + 1].to_broadcast((P, SEQ)),
                    mybir.AluOpType.add,
                )
            else:
                nc.scalar.activation(
                    outbuf[:, m, rs], ps[:], AF.Identity, bias=b2[:, m : m + 1]
                )

        def scores_kchunk(b, k, pss):
            # one k-slice of the scores accumulation: needs only the m=k tiles
            # of headWT/depT, so it can sit right after the m=k+1 layer-2 pair
            js = slice(b * SEQ, (b + 1) * SEQ)
            for i in range(AK):
                nc.tensor.matmul(
                    pss[i][:],
                    headWT[:, k, b * SEQ + i * P : b * SEQ + (i + 1) * P],
                    depT[:, k, js],
                    start=(k == 0),
                    stop=(k == AK - 1),
                )

        def scores_out(b, pss):
            # the bilinear bias bb is carried by arc pad row 500
            # (headWT[500,:] = bb via the fused bias, depT[500,:] = 1.0), so
            # the PSUM result is final: plain copy out, alternating engines
            for i in range(AK):
                ot = opool.tile([P, SEQ], F32, tag="scout")
                if i % 2 == 0:
                    nc.vector.tensor_copy(ot[:], pss[i][:])
                else:
                    # Identity (not Copy) keeps the ACT function table unchanged
                    # from the depT bias-adds -- table reloads are expensive
                    nc.scalar.activation(ot[:], pss[i][:], AF.Identity)
                eng = nc.sync if i % 2 == 0 else nc.scalar
                eng.dma_start(aps["scores"][b, i * P : (i + 1) * P, :], ot[:])

        if loop_n:
            hints = _cfg("loop_hints", ())
            if hints == "all":
                hints = tuple(
                    mybir.EngineType(e)
                    for e in ("PE", "Activation", "DVE", "SP", "Pool")
                )
            loop_cm = tc.For_i(0, loop_n, 1, hint_engines=hints)
        else:
            loop_cm = contextlib.nullcontext()
        if _cfg("tiny_body", False) and loop_n:
            with loop_cm:
                tb = apool.tile([P, 16], F32, tag="tinybody")
                nc.vector.tensor_copy(tb[:], biases[:, 0:16])
            return
        with loop_cm:
            # both layer-1s first (independent), so layer-2 never starves PE;
            # rc-halves in DMA-arrival order
            l1_half(w1h, b1h, h1h, 0)
            l1_half(w1d, b1d, h1d, 0)
            l1_half(w1h, b1h, h1h, 1)
            l1_half(w1d, b1d, h1d, 1)
            # layer 2 + scores software-pipelined per batch-half: the scores
            # k-chunk for m=k is emitted after the m=k+1 layer-2 pair, so the
            # in-order PE stream never waits on an epilogue drain.
            for rc in range(B_PER_CORE):
                pss = []
                for i in range(AK):
                    ps = pspool.tile([P, SEQ], F32, tag="ps", name=f"ps2s_{rc}_{i}")
                    pss.append(ps)
                for m in range(AK):
                    l2_group(wf, bfh, h1h, headWT, rc, m, "dve")
                    l2_group(w2d, b2d, h1d, depT, rc, m, "act")
                    if m >= 1:
                        scores_kchunk(rc, m - 1, pss)
                scores_kchunk(rc, AK - 1, pss)
                scores_out(rc, pss)


def _build(loop_n=0):
    sd, md = _DT_MODES[_cfg("dt_mode", "f32r")]
    key = ("nc", _cfg("dt_mode", "f32r"), _cfg("scores_f32r", True), loop_n, _cfg("loop_hints", ()), _cfg("tiny_body", False))
    if key in _CACHE:
        return _CACHE[key]
    nc = bacc.Bacc("TRN2", target_bir_lowering=False, debug=False, num_devices=N_CORES)

    def dram(name, shape, dt):
        return nc.dram_tensor(name, shape, dt, kind="ExternalInput").ap()

    aps = {
        "xT": dram("xT", [HIDDEN, R], md),
        "w1hT": dram("w1hT", [HIDDEN, ARC_P], sd),
        "wfT": dram("wfT", [ARC_P, ARC_P], sd),
        "w1dT": dram("w1dT", [HIDDEN, ARC_P], sd),
        "w2dT": dram("w2dT", [ARC_P, ARC_P], sd),
        "biasesL": dram("biasesL", [P, 4 * AK + 1], F32),
        "scores": nc.dram_tensor(
            "scores", [B_PER_CORE, SEQ, SEQ], F32, kind="ExternalOutput"
        ).ap(),
    }
    with tile.TileContext(nc) as tc:
        _emit(nc, tc, aps, loop_n=loop_n)
    nc.compile()
    _CACHE[key] = nc
    return nc


def _round_tf32(a):
    """fp32 -> tf32 (10-bit mantissa) RNE, returned as fp32 bits."""
    b = np.ascontiguousarray(a, np.float32).view(np.uint32).copy()
    lsb = (b >> 13) & 1
    b += 0x0FFF + lsb
    b &= np.uint32(0xFFFFE000)
    return b.view(np.float32)


def _to_dt(a, dt):
    """Convert fp32 ndarray to the numpy repr of mybir dtype dt."""
    if dt == F32R:
        return _round_tf32(a)
    if dt == BF16:
        import ml_dtypes

        return np.asarray(a, np.float32).astype(ml_dtypes.bfloat16)
    if dt == F16:
        return np.asarray(a, np.float32).astype(np.float16)
    return np.asarray(a, np.float32)


def _bias_layout(b):
    """[ARC] (unpadded) -> [128, AK] with arc index = col*128 + partition."""
    bp = np.zeros(ARC_P, np.float32)
    b = np.asarray(b, np.float32)
    bp[: b.shape[0]] = b
    return np.ascontiguousarray(bp.reshape(AK, P).T)


def _prep_shared(w1h, b1h, w2h, b2h, w1d, b1d, w2d, b2d, Wb, bb):
    sd, _ = _DT_MODES[_cfg("dt_mode", "f32r")]

    def padT(w, rows, cols):
        """Pad w.T (fp32/64 in) to [rows, cols], convert to stationary dtype."""
        out = np.zeros((rows, cols), np.float32)
        wt = np.asarray(w, np.float64).T
        out[: wt.shape[0], : wt.shape[1]] = wt.astype(np.float32)
        return _to_dt(out, sd)

    # fuse Wb into head layer 2 (float64 on host):
    # head @ Wb = relu(x@w1h.T+b1h) @ (w2h.T @ Wb) + (b2h @ Wb)
    wf = np.asarray(w2h, np.float64).T @ np.asarray(Wb, np.float64)  # [arc1, arc2]
    bf = np.asarray(b2h, np.float64) @ np.asarray(Wb, np.float64)  # [arc2]
    # carry the bilinear bias bb through arc pad row 500: headWT[500,:] = bb
    # (bias-only row: pad weight columns are zero), depT[500,:] = 1.0, so the
    # 512-wide scores contraction contributes bb * 1 exactly.
    bf = np.concatenate([bf, [float(np.asarray(bb).reshape(-1)[0])]])
    b2d_aug = np.concatenate([np.asarray(b2d, np.float64), [1.0]])
    return {
        "w1hT": padT(w1h, HIDDEN, ARC_P),
        "wfT": padT(wf.T, ARC_P, ARC_P),  # padT transposes back -> [arc1, arc2]
        "w1dT": padT(w1d, HIDDEN, ARC_P),
        "w2dT": padT(w2d, ARC_P, ARC_P),
        "biasesL": np.concatenate(
            [
                _bias_layout(b1h),
                _bias_layout(bf.astype(np.float32)),
                _bias_layout(b1d),
                _bias_layout(b2d_aug.astype(np.float32)),
                np.full((P, 1), float(np.asarray(bb).reshape(-1)[0]), np.float32),
            ],
            axis=1,
        ),
    }


def kernel(hidden_states, w1h, b1h, w2h, b2h, w1d, b1d, w2d, b2d, Wb, bb):
    import time

    _, md = _DT_MODES[_cfg("dt_mode", "f32r")]
    nc = _build(loop_n=int(_cfg("loop_n", 0)))
    shared = _prep_shared(w1h, b1h, w2h, b2h, w1d, b1d, w2d, b2d, Wb, bb)
    x = np.asarray(hidden_states, np.float32)
    in_maps = []
    for c in range(N_CORES):
        xc = x[c * B_PER_CORE : (c + 1) * B_PER_CORE].reshape(R, HIDDEN)
        in_maps.append({"xT": _to_dt(np.ascontiguousarray(xc.T), md), **shared})
    t0 = time.perf_counter()
    res = run_bass_kernel_spmd(nc, in_maps, core_ids=list(range(N_CORES)))
    _CACHE["last_run_seconds"] = time.perf_counter() - t0
    out = np.empty((BATCH, SEQ, SEQ), np.float32)
    for c in range(N_CORES):
        out[c * B_PER_CORE : (c + 1) * B_PER_CORE] = res.results[c]["scores"]
    return out


def _selftest():
    rng = np.random.default_rng(0)
    s_h = 1.0 / np.sqrt(HIDDEN)
    s_a = 1.0 / np.sqrt(ARC)
    ins = {
        "hidden_states": rng.standard_normal((BATCH, SEQ, HIDDEN)).astype(np.float32),
        "w1h": rng.uniform(-s_h, s_h, (ARC, HIDDEN)).astype(np.float32),
        "b1h": rng.uniform(-s_h, s_h, (ARC,)).astype(np.float32),
        "w2h": rng.uniform(-s_a, s_a, (ARC, ARC)).astype(np.float32),
        "b2h": rng.uniform(-s_a, s_a, (ARC,)).astype(np.float32),
        "w1d": rng.uniform(-s_h, s_h, (ARC, HIDDEN)).astype(np.float32),
        "b1d": rng.uniform(-s_h, s_h, (ARC,)).astype(np.float32),
        "w2d": rng.uniform(-s_a, s_a, (ARC, ARC)).astype(np.float32),
        "b2d": rng.uniform(-s_a, s_a, (ARC,)).astype(np.float32),
        "Wb": rng.uniform(-s_a, s_a, (ARC, ARC)).astype(np.float32),
        "bb": rng.uniform(-s_a, s_a, (1,)).astype(np.float32),
    }
    out = kernel(**ins)

    def ref_mlp(x, w1, b1, w2, b2):
        return np.maximum(x @ w1.T + b1, 0.0) @ w2.T + b2

    head = ref_mlp(ins["hidden_states"], ins["w1h"], ins["b1h"], ins["w2h"], ins["b2h"])
    dep = ref_mlp(ins["hidden_states"], ins["w1d"], ins["b1d"], ins["w2d"], ins["b2d"])
    headW = head @ ins["Wb"]
    exp = np.einsum("bia,bja->bij", headW, dep) + ins["bb"][0]
    err = np.abs(out - exp)
    rel = err.max() / np.abs(exp).max()
    print(f"max abs err {err.max():.3e}  absmax-rel {rel:.3e}")
    print(f"run seconds: {_CACHE.get('last_run_seconds'):.3f}")


if __name__ == "__main__":
    for mode in sys.argv[1:] or ["fp16"]:
        _CACHE.clear()
        _CACHE["dt_mode"] = mode
        print(f"--- dt_mode={mode}")
        _selftest()



# revision 28
# speedup vs baseline: 1.5958x; 1.5958x over previous
"""BiaffineAttention TRN2 kernel.

Full-input contract: kernel(**inputs) takes the unsharded reference inputs
(hidden_states [16,512,1024] f32 + MLP/bilinear params) and returns the full
arc_scores [16,512,512] f32.

Strategy:
- Data-parallel over batch across 8 NeuronCores (2 batches/core).
- All on-chip compute is feature-major (arc/hidden on partitions): every
  matmul contracts over partitions, no on-chip transposes.
- The whole dep-side second linear layer is eliminated algebraically:
      scores = (Hh@Wf + bf) @ (Hd@w2d^T + b2d)^T + bb
             = Hh @ G @ Hd^T + 1*(u.Hd^T) + (Hh.v)*1^T + c
  with G = Wf@w2d, u = bf@w2d, v = Wf@b2d, c = bf.b2d + bb (all folded on
  the host in float64, Wf = w2h^T@Wb, bf = b2h@Wb).  The rank-1 terms ride
  for free: u becomes the bias of the A = Hh@G epilogue, (Hh.v + c) lands in
  arc pad column 500 of A (G column 500 = v, bias col = c), and Hd's pad
  row 500 is forced to 1.0 via its layer-1 bias.  Device GEMMs per core:
  2x L1 (K=1024), A (K=512), scores (K=512) = 98304 PE cycles (was 114688).
- Inputs are packed into one striped DRAM tensor TIN[k, xT-b0|w1h|xT-b1|w1d]
  so the input stream needs only ~1.5 descriptor generations per L1 k-step,
  and the first k-tile lands in two parallel half-stripe DMAs.
- Matmuls run in float16 (10-bit mantissa, 2-byte operands at full PE rate).
"""

import sys

if "/opt/trn_rl_repo" not in sys.path:
    sys.path.insert(0, "/opt/trn_rl_repo")

import numpy as np

import concourse.bacc as bacc
import concourse.mybir as mybir
import concourse.tile as tile
from concourse.bass_utils import run_bass_kernel_spmd

N_CORES = 8
BATCH = 16
SEQ = 512
HIDDEN = 1024
ARC = 500
ARC_P = 512  # arc padded to a multiple of 128

P = 128
B_PER_CORE = BATCH // N_CORES  # 2
R = B_PER_CORE * SEQ  # 1024 rows per core
HK = HIDDEN // P  # 8 hidden k-tiles
AK = ARC_P // P  # 4 arc tiles
CW = 4 * SEQ  # TIN stripe width: xT-b0 | w1h | xT-b1 | w1d

F32 = mybir.dt.float32
F16 = mybir.dt.float16
AF = mybir.ActivationFunctionType
ADD = mybir.AluOpType.add
MAX = mybir.AluOpType.max

_CACHE = {}
_DEFAULTS = {"warm_n": 200, "s1_rings": "ysy", "jh_rings": "yg"}


def _cfg(name, default=None):
    return _CACHE.get(name, _DEFAULTS.get(name, default))


def _emit(nc, tc, aps, loop_n=0):
    import contextlib

    ctx = contextlib.ExitStack()
    with ctx:
        cpool = ctx.enter_context(tc.tile_pool(name="const", bufs=1))
        apool = ctx.enter_context(tc.tile_pool(name="acts", bufs=1))
        pspool = ctx.enter_context(tc.tile_pool(name="psum", bufs=8, space="PSUM"))
        opool = ctx.enter_context(tc.tile_pool(name="outs", bufs=8))

        # ---- resident SBUF tensors
        xw = cpool.tile([P, HK, CW], F16, tag="xw")  # xT-b0 | w1h | xT-b1 | w1d
        g = cpool.tile([P, AK, ARC_P], F16, tag="g")  # Gaug (arc1 parts, arc2)
        biases = cpool.tile([P, 3 * AK], F32, tag="biases")
        b1h = biases[:, 0 * AK : 1 * AK]
        uB = biases[:, 1 * AK : 2 * AK]
        b1d = biases[:, 2 * AK : 3 * AK]

        h1h = apool.tile([P, AK, R], F16, tag="h1h")
        h1d = apool.tile([P, AK, R], F16, tag="h1d")
        aT = apool.tile([P, AK, R], F16, tag="aT")

        # ---- PE warm-up: a chain of tiny self-contained matmuls keeps the
        # Tensor engine "busy" from ~0.7us so the real matmuls (whose first
        # operands land at ~3.6us) are issued past the p-state ramp window
        # and run at full clock.  The chain ends right around data arrival.
        warm_n = int(_cfg("warm_n", 225))
        if warm_n:
            wdum = cpool.tile([P, P], F16, tag="wdum")
            nc.vector.memset(wdum[:], 0.0)
            wps = pspool.tile([P, SEQ], F32, tag="ps", name="warm")
            for _ in range(warm_n):
                nc.tensor.matmul(wps[:, 0:16], wdum[:], wdum[:, 0:16], start=True, stop=True)

        # ---- input DMA schedule.  TIN stripe k = [xT-b0 | w1h | w1d | xT-b1]
        # for hidden k-tile k.  P0 consumes only cols 0:3*SEQ, so the head
        # stream sends those (1092ns/stripe vs 1704ns of PE work per stripe)
        # in consumption order; xT-b1 follows as one bulk transfer and the
        # Gaug weights last (needed only by the A phase).
        tin = aps["tin"].rearrange("(ko p) c -> p ko c", p=P)
        nc.sync.dma_start(xw[:, 0, 0 : 2 * SEQ], tin[:, 0, 0 : 2 * SEQ])
        nc.gpsimd.dma_start(xw[:, 0, 2 * SEQ : 3 * SEQ], tin[:, 0, 2 * SEQ : 3 * SEQ])
        nc.scalar.dma_start(xw[:, 1, 0 : 2 * SEQ], tin[:, 1, 0 : 2 * SEQ])
        nc.gpsimd.dma_start(xw[:, 1, 2 * SEQ : 3 * SEQ], tin[:, 1, 2 * SEQ : 3 * SEQ])
        for k in range(2, HK):
            nc.sync.dma_start(xw[:, k], tin[:, k])
        nc.scalar.dma_start(biases[:], aps["biasesL"])
        nc.gpsimd.dma_start(xw[:, 0:2, 3 * SEQ : CW], tin[:, 0:2, 3 * SEQ : CW])
        nc.sync.dma_start(g[:], aps["g"].rearrange("(ko p) a -> p ko a", p=P))

        def l1_block(k, woff, rs, pss):
            # pss[m] += w1[:, k, m-slice].T @ xT[:, k, rs]
            for m in range(AK):
                nc.tensor.matmul(
                    pss[m][:],
                    xw[:, k, woff + m * P : woff + (m + 1) * P],
                    xw[:, k, rs],
                    start=(k == 0),
                    stop=(k == HK - 1),
                )

        def l1_phase(rc):
            # both L1s for batch-half rc, interleaved per k so the head
            # phase needs only one arriving stripe per 8 matmuls
            rs = slice(0, SEQ) if rc == 0 else slice(3 * SEQ, CW)
            ph = [pspool.tile([P, SEQ], F32, tag="ps", name=f"l1h_{rc}_{m}") for m in range(AK)]
            pd = [pspool.tile([P, SEQ], F32, tag="ps", name=f"l1d_{rc}_{m}") for m in range(AK)]
            for k in range(HK):
                l1_block(k, SEQ, rs, ph)
                l1_block(k, 2 * SEQ, rs, pd)
            os = slice(rc * SEQ, (rc + 1) * SEQ)
            # h-epilogues split ACT/DVE so all four finish ~1.3us after the
            # last h matmul (the A-phase consumes them k2-by-k2)
            for m in range(AK):
                if m % 2 == 0:
                    nc.scalar.activation(h1h[:, m, os], ph[m][:], AF.Relu, bias=b1h[:, m : m + 1])
                else:
                    nc.vector.tensor_tensor(h1h[:, m, os], ph[m][:], b1h[:, m : m + 1].to_broadcast((P, SEQ)), ADD)
                    nc.vector.tensor_scalar_max(h1h[:, m, os], h1h[:, m, os], 0.0)
            # d-epilogues all on ACT (needed one A-phase later)
            for m in range(AK):
                nc.scalar.activation(h1d[:, m, os], pd[m][:], AF.Relu, bias=b1d[:, m : m + 1])

        def a_phase(rc):
            # aT[:, m, rc] = (Hh @ Gaug).T + u  (bias add per arc2 partition)
            rs = slice(rc * SEQ, (rc + 1) * SEQ)
            for m in range(AK):
                ps = pspool.tile([P, SEQ], F32, tag="ps", name=f"a_{rc}_{m}")
                for k2 in range(AK):
                    nc.tensor.matmul(
                        ps[:],
                        g[:, k2, m * P : (m + 1) * P],
                        h1h[:, k2, rs],
                        start=(k2 == 0),
                        stop=(k2 == AK - 1),
                    )
                # alternate DVE/ACT so the last tile's epilogue lands early
                if m % 2 == 0:
                    nc.vector.tensor_tensor(aT[:, m, rs], ps[:], uB[:, m : m + 1].to_broadcast((P, SEQ)), ADD)
                else:
                    nc.scalar.activation(aT[:, m, rs], ps[:], AF.Identity, bias=uB[:, m : m + 1])

        def s_phase(rc, last=False):
            # scores[rc][i-block] = aT-slice.T @ h1d ; i-outer so each output
            # tile drains (copy + DMA) while the next accumulates.  Early
            # tiles go out via the SWDGE ring (separate descriptor-gen unit);
            # the final tile of the kernel is j-split so its copy + DMA chain
            # after the last matmul is as short as possible.
            rs = slice(rc * SEQ, (rc + 1) * SEQ)
            H = SEQ // 2
            for i in range(AK):
                if last and i == AK - 1:
                    # j-split the final tile [384 | 128]; the first piece's DMA
                    # descriptor-gen goes to the otherwise-idle SWDGE unit so
                    # the last piece's HWDGE gen starts the moment its copy
                    # lands, on a ring whose sequencer is parked waiting on it
                    for jh, (j0, j1) in enumerate(((0, 3 * P), (3 * P, SEQ))):
                        w = j1 - j0
                        ps = pspool.tile([P, SEQ], F32, tag="ps", name=f"s_{rc}_{i}_{jh}")
                        js = slice(rc * SEQ + j0, rc * SEQ + j1)
                        for k2 in range(AK):
                            nc.tensor.matmul(
                                ps[:, 0:w],
                                aT[:, k2, rc * SEQ + i * P : rc * SEQ + (i + 1) * P],
                                h1d[:, k2, js],
                                start=(k2 == 0),
                                stop=(k2 == AK - 1),
                            )
                        ot = opool.tile([P, w], F16, tag=f"scout_{jh}")
                        if jh == 0:
                            nc.scalar.activation(ot[:], ps[:, 0:w], AF.Identity)
                        else:
                            nc.vector.tensor_copy(ot[:], ps[:, 0:w])
                        jr = _cfg("jh_rings", "gy")
                        eng = {"y": nc.sync, "s": nc.scalar, "g": nc.gpsimd}[jr[jh]]
                        eng.dma_start(
                            aps["scores"][rc, i * P : (i + 1) * P, j0:j1],
                            ot[:],
                        )
                    continue
                ps = pspool.tile([P, SEQ], F32, tag="ps", name=f"s_{rc}_{i}")
                for k2 in range(AK):
                    nc.tensor.matmul(
                        ps[:],
                        aT[:, k2, rc * SEQ + i * P : rc * SEQ + (i + 1) * P],
                        h1d[:, k2, rs],
                        start=(k2 == 0),
                        stop=(k2 == AK - 1),
                    )
                ot = opool.tile([P, SEQ], F16, tag="scout")
                if i % 2 == 0:
                    nc.vector.tensor_copy(ot[:], ps[:])
                else:
                    nc.scalar.activation(ot[:], ps[:], AF.Identity)
                if last:
                    rings = _cfg("s1_rings", "ysg")  # rings for i0,i1,i2
                    eng = {"y": nc.sync, "s": nc.scalar, "g": nc.gpsimd}[rings[i]]
                else:
                    eng = nc.sync if i % 2 == 0 else nc.scalar
                eng.dma_start(aps["scores"][rc, i * P : (i + 1) * P, :], ot[:])

        if loop_n:
            hints = _cfg("loop_hints", ())
            if hints == "all":
                hints = tuple(
                    mybir.EngineType(e) for e in ("PE", "Activation", "DVE", "SP", "Pool")
                )
            loop_cm = tc.For_i(0, loop_n, 1, hint_engines=hints)
        else:
            loop_cm = contextlib.nullcontext()
        if _cfg("tiny_body", False) and loop_n:
            with loop_cm:
                tb = apool.tile([P, 16], F32, tag="tinybody")
                nc.vector.tensor_copy(tb[:], biases[:, 0:16])
            return
        with loop_cm:
            l1_phase(0)
            a_phase(0)
            # first k-step of P1's head half fills the aT-epilogue seam
            rs1 = slice(3 * SEQ, CW)
            os1 = slice(SEQ, 2 * SEQ)
            ph1 = [pspool.tile([P, SEQ], F32, tag="ps", name=f"l1h_1_{m}") for m in range(AK)]
            l1_block(0, SEQ, rs1, ph1)
            s_phase(0)
            for k in range(1, HK):
                l1_block(k, SEQ, rs1, ph1)
            for m in range(AK):
                if m % 2 == 0:
                    nc.scalar.activation(h1h[:, m, os1], ph1[m][:], AF.Relu, bias=b1h[:, m : m + 1])
                else:
                    nc.vector.tensor_tensor(h1h[:, m, os1], ph1[m][:], b1h[:, m : m + 1].to_broadcast((P, SEQ)), ADD)
                    nc.vector.tensor_scalar_max(h1h[:, m, os1], h1h[:, m, os1], 0.0)
            pd1 = [pspool.tile([P, SEQ], F32, tag="ps", name=f"l1d_1_{m}") for m in range(AK)]
            for k in range(HK):
                l1_block(k, 2 * SEQ, rs1, pd1)
            for m in range(AK):
                nc.scalar.activation(h1d[:, m, os1], pd1[m][:], AF.Relu, bias=b1d[:, m : m + 1])
            a_phase(1)
            s_phase(1, last=True)


def _build(loop_n=0):
    key = ("nc", loop_n, _cfg("loop_hints", ()), _cfg("tiny_body", False),
           _cfg("warm_n", 225), _cfg("s1_rings", "ysg"), _cfg("jh_rings", "gy"))
    if key in _CACHE:
        return _CACHE[key]
    nc = bacc.Bacc("TRN2", target_bir_lowering=False, debug=False, num_devices=N_CORES)

    def dram(name, shape, dt):
        return nc.dram_tensor(name, shape, dt, kind="ExternalInput").ap()

    aps = {
        "tin": dram("tin", [HIDDEN, CW], F16),
        "g": dram("g", [ARC_P, ARC_P], F16),
        "biasesL": dram("biasesL", [P, 3 * AK], F32),
        "scores": nc.dram_tensor(
            "scores", [B_PER_CORE, SEQ, SEQ], F16, kind="ExternalOutput"
        ).ap(),
    }
    with tile.TileContext(nc) as tc:
        _emit(nc, tc, aps, loop_n=loop_n)
    nc.compile()
    _CACHE[key] = nc
    return nc


def _bias_layout(b):
    """[<=512] -> [128, AK] with arc index = col*128 + partition."""
    bp = np.zeros(ARC_P, np.float32)
    b = np.asarray(b, np.float32)
    bp[: b.shape[0]] = b
    return np.ascontiguousarray(bp.reshape(AK, P).T)


def _prep_shared(w1h, b1h, w2h, b2h, w1d, b1d, w2d, b2d, Wb, bb):
    f8 = np.float64
    w2h, b2h, w2d, b2d, Wb = (np.asarray(a, f8) for a in (w2h, b2h, w2d, b2d, Wb))
    bb0 = float(np.asarray(bb).reshape(-1)[0])
    Wf = w2h.T @ Wb  # [arc1, arc2]
    bf = b2h @ Wb  # [arc2]
    G = Wf @ w2d  # [arc1, arcd]
    u = bf @ w2d  # [arcd]
    v = Wf @ b2d  # [arc1]
    c = float(bf @ b2d) + bb0

    Gaug = np.zeros((ARC_P, ARC_P), np.float32)
    Gaug[:ARC, :ARC] = G
    Gaug[:ARC, ARC] = v
    u_aug = np.zeros(ARC_P, np.float64)
    u_aug[:ARC] = u
    u_aug[ARC] = c
    b1d_aug = np.zeros(ARC_P, np.float64)
    b1d_aug[:ARC] = np.asarray(b1d, f8)
    b1d_aug[ARC] = 1.0  # Hd pad column 500 = relu(0*x + 1) = 1

    def padT(w):
        out = np.zeros((HIDDEN, ARC_P), np.float32)
        wt = np.asarray(w, f8).T
        out[: wt.shape[0], : wt.shape[1]] = wt
        return out.astype(np.float16)

    return {
        "w1hT": padT(w1h),
        "w1dT": padT(w1d),
        "g": Gaug.astype(np.float16),
        "biasesL": np.concatenate(
            [
                _bias_layout(b1h),
                _bias_layout(u_aug.astype(np.float32)),
                _bias_layout(b1d_aug.astype(np.float32)),
            ],
            axis=1,
        ),
    }


def kernel(hidden_states, w1h, b1h, w2h, b2h, w1d, b1d, w2d, b2d, Wb, bb):
    import time

    nc = _build(loop_n=int(_cfg("loop_n", 0)))
    shared = _prep_shared(w1h, b1h, w2h, b2h, w1d, b1d, w2d, b2d, Wb, bb)
    x = np.asarray(hidden_states, np.float32)
    in_maps = []
    for c in range(N_CORES):
        xc = x[c * B_PER_CORE : (c + 1) * B_PER_CORE].reshape(R, HIDDEN)
        xT = np.ascontiguousarray(xc.T).astype(np.float16)  # [HIDDEN, R]
        tin = np.empty((HIDDEN, CW), np.float16)
        tin[:, 0:SEQ] = xT[:, 0:SEQ]
        tin[:, SEQ : 2 * SEQ] = shared["w1hT"]
        tin[:, 2 * SEQ : 3 * SEQ] = shared["w1dT"]
        tin[:, 3 * SEQ : CW] = xT[:, SEQ:R]
        in_maps.append({"tin": tin, "g": shared["g"], "biasesL": shared["biasesL"]})
    t0 = time.perf_counter()
    res = run_bass_kernel_spmd(nc, in_maps, core_ids=list(range(N_CORES)))
    _CACHE["last_run_seconds"] = time.perf_counter() - t0
    out = np.empty((BATCH, SEQ, SEQ), np.float32)
    for c in range(N_CORES):
        out[c * B_PER_CORE : (c + 1) * B_PER_CORE] = np.asarray(res.results[c]["scores"], np.float32)
    return out


def _selftest():
    rng = np.random.default_rng(0)
    s_h = 1.0 / np.sqrt(HIDDEN)
    s_a = 1.0 / np.sqrt(ARC)
    ins = {
        "hidden_states": rng.standard_normal((BATCH, SEQ, HIDDEN)).astype(np.float32),
        "w1h": rng.uniform(-s_h, s_h, (ARC, HIDDEN)).astype(np.float32),
        "b1h": rng.uniform(-s_h, s_h, (ARC,)).astype(np.float32),
        "w2h": rng.uniform(-s_a, s_a, (ARC, ARC)).astype(np.float32),
        "b2h": rng.uniform(-s_a, s_a, (ARC,)).astype(np.float32),
        "w1d": rng.uniform(-s_h, s_h, (ARC, HIDDEN)).astype(np.float32),
        "b1d": rng.uniform(-s_h, s_h, (ARC,)).astype(np.float32),
        "w2d": rng.uniform(-s_a, s_a, (ARC, ARC)).astype(np.float32),
        "b2d": rng.uniform(-s_a, s_a, (ARC,)).astype(np.float32),
        "Wb": rng.uniform(-s_a, s_a, (ARC, ARC)).astype(np.float32),
        "bb": rng.uniform(-s_a, s_a, (1,)).astype(np.float32),
    }
    out = kernel(**ins)

    def ref_mlp(x, w1, b1, w2, b2):
        return np.maximum(x @ w1.T + b1, 0.0) @ w2.T + b2

    head = ref_mlp(ins["hidden_states"], ins["w1h"], ins["b1h"], ins["w2h"], ins["b2h"])
    dep = ref_mlp(ins["hidden_states"], ins["w1d"], ins["b1d"], ins["w2d"], ins["b2d"])
    headW = head @ ins["Wb"]
    exp = np.einsum("bia,bja->bij", headW, dep) + ins["bb"][0]
    err = np.abs(out - exp)
    rel = err.max() / np.abs(exp).max()
    print(f"max abs err {err.max():.3e}  absmax-rel {rel:.3e}")
    print(f"run seconds: {_CACHE.get('last_run_seconds'):.3f}")


if __name__ == "__main__":
    _selftest()
